# revision 19
# baseline (speedup 1.0000x reference)
"""Fused top-k/top-p/sampling kernel for Trainium2 (8 NeuronCores).

Contract: kernel(**inputs) takes FULL inputs (logits [256,128000] f32,
top_ks [256] int, top_ps [256] f32, q [256,128000] f32) and returns the
FULL output tuple (selected_idx [256] int32, selected_logits [256,128000]
f32), matching reference semantics.

Strategy (rows sharded 32/core across 8 cores, pure data parallel):
  Phase 1 (device): per row, exact top-256 (values + indices) via the
      gpsimd `topk` ucode instruction. Only ~16KB/core comes back.
  Host glue (O(rows*64)): stable top-64 order, replicate the reference's
      f32 top-k/top-p prefix decision on the 64-wide window, derive the
      per-row value cutoff v_cut and the sampled index (argmax of
      p/(q+eps) over the <=63 kept positions, gathering q on host).
  Phase 2 (device): stream logits again and write
      out = x + 2*((x < v_cut) * -1.8e38)
      which is exactly x where x >= v_cut and exactly -inf elsewhere
      (the *2 overflows -3.6e38 to -inf in f32; 0*2+x == x is exact).
      Tie rows (elements == v_cut beyond the kept prefix) are patched on
      host using the exact candidate info from phase 1.

The decision math only needs the top-63 elements per row (top_ks < 64),
so q is never streamed on device and the device does two 16MB/core reads
plus one 16MB/core write: ~393MB of HBM traffic total across 8 cores.
"""

import os
import sys

if "/opt/trn_rl_repo" not in sys.path:
    sys.path.insert(0, "/opt/trn_rl_repo")

import numpy as np

R, V = 256, 128000
NCORES = 8
RPC = R // NCORES          # 32 rows per core
HALVES = 2                 # topk ISA encodes n as u16 -> split rows in half
VH = V // HALVES           # 64000 per half-row "token"
VROWS = RPC * HALVES       # 64 virtual tokens per core
TOKS = 8                   # tokens per topk instruction
CALLS = VROWS // TOKS      # 8 topk calls per core
KTOP = 256                 # candidates per half-row from device
TOPQ = 64                  # candidates actually used per row (top_ks < 64)
F2 = 2000                  # phase-2 free-dim tile size (per partition)
NCH = V // 4 // F2         # 16 chunks (4 partitions per row)
BIG = -1.8e38              # (x<t)*BIG*2 + x  ->  -inf exactly when dropped
EPS = 1e-08
NEG_INF = np.float32(-np.inf)

_cache = {}
last_exec_ns = {}          # phase -> exec_time_ns (filled when tracing)


def _build_phase1():
    from contextlib import ExitStack

    import concourse.mybir as mybir
    from concourse import bacc

    nc = bacc.Bacc()
    x = nc.dram_tensor("x", [RPC, V], mybir.dt.float32, kind="ExternalInput")
    cand = nc.dram_tensor(
        "cand", [CALLS, 128, 32], mybir.dt.uint32, kind="ExternalOutput"
    )
    with (
        nc.semaphore("dma_a") as dma_a,
        nc.semaphore("dma_b") as dma_b,
        nc.semaphore("dma_c") as dma_c,
        nc.semaphore("comp") as comp,
        nc.sbuf_tensor("xb0", [128, VH // 16], mybir.dt.float32) as xb0,
        nc.sbuf_tensor("xb1", [128, VH // 16], mybir.dt.float32) as xb1,
        ExitStack() as stack,
    ):
        cb = [
            stack.enter_context(
                nc.sbuf_tensor(f"cb{t}", [128, 32], mybir.dt.uint32)
            )
            for t in range(CALLS)
        ]
        xbufs = [xb0, xb1]
        in_sems = [dma_a, dma_b]
        # rows x halves as 64 virtual tokens of width VH (contiguous)
        xv = x[:, :].rearrange("r (h n) -> (r h) n", h=HALVES)

        with nc.Block() as block:

            @block.sync
            def _(sync):
                for t in range(CALLS):
                    if t >= 2:
                        # buffer t%2 is free once topk t-2 completed
                        sync.wait_ge(comp, t - 1)
                    sync.dma_start(
                        xbufs[t % 2][:, :],
                        xv[t * TOKS : (t + 1) * TOKS, :].rearrange(
                            "v (p c) -> (v p) c", p=16
                        ),
                    ).then_inc(in_sems[t % 2], 16)
                for t in range(CALLS):
                    sync.wait_ge(comp, t + 1)
                    sync.dma_start(cand[t, :, :], cb[t][:, :]).then_inc(dma_c, 16)
                sync.wait_ge(dma_a, 16 * (CALLS // 2))
                sync.wait_ge(dma_b, 16 * (CALLS // 2))
                sync.wait_ge(dma_c, 16 * CALLS)

            @block.gpsimd
            def _(g):
                for t in range(CALLS):
                    g.wait_ge(in_sems[t % 2], 16 * (t // 2 + 1))
                    g.topk(
                        cb[t][:, :],
                        xbufs[t % 2][:, :],
                        tokens=TOKS,
                        vocab_size=VH,
                        k=KTOP,
                    ).then_inc(comp, 1)

    nc.finalize()
    return nc


def _build_fused():
    """Single launch: topk candidates + full -inf output materialization.

    The -inf fill (the entire output payload) has no data dependency on
    the input, so its DMA writes overlap the logits reads and the topk
    scans. The host later scatters the <=63 kept logits per row into the
    returned buffer (an O(rows*64) patch, same order as the decision
    glue itself).
    """
    from contextlib import ExitStack

    import concourse.mybir as mybir
    from concourse import bacc

    FW = 4000  # fill tile free size -> 8 chunks of [128, FW] cover a core
    NFCH = V * RPC // (128 * FW)

    nc = bacc.Bacc()
    x = nc.dram_tensor("x", [RPC, V], mybir.dt.float32, kind="ExternalInput")
    cand = nc.dram_tensor(
        "cand", [CALLS, 128, 32], mybir.dt.uint32, kind="ExternalOutput"
    )
    y = nc.dram_tensor("y", [RPC, V], mybir.dt.float32, kind="ExternalOutput")

    with (
        nc.semaphore("dma_a") as dma_a,
        nc.semaphore("dma_b") as dma_b,
        nc.semaphore("dma_c") as dma_c,
        nc.semaphore("dma_y") as dma_y,
        nc.semaphore("comp") as comp,
        nc.semaphore("msem") as msem,
        nc.sbuf_tensor("xb0", [128, VH // 16], mybir.dt.float32) as xb0,
        nc.sbuf_tensor("xb1", [128, VH // 16], mybir.dt.float32) as xb1,
        nc.sbuf_tensor("cinf", [128, FW], mybir.dt.float32) as cinf,
        ExitStack() as stack,
    ):
        cb = [
            stack.enter_context(
                nc.sbuf_tensor(f"cb{t}", [128, 32], mybir.dt.uint32)
            )
            for t in range(CALLS)
        ]
        xbufs = [xb0, xb1]
        in_sems = [dma_a, dma_b]
        xv = x[:, :].rearrange("r (h n) -> (r h) n", h=HALVES)
        # partition p = 4*r + quarter; 32000 contiguous elements each
        yf = y[:, :].rearrange("r (a n) -> (r a) n", a=4)

        with nc.Block() as block:

            @block.sync
            def _(sync):
                for t in range(CALLS):
                    if t >= 2:
                        sync.wait_ge(comp, t - 1)
                    sync.dma_start(
                        xbufs[t % 2][:, :],
                        xv[t * TOKS : (t + 1) * TOKS, :].rearrange(
                            "v (p c) -> (v p) c", p=16
                        ),
                    ).then_inc(in_sems[t % 2], 16)
                for t in range(CALLS):
                    sync.wait_ge(comp, t + 1)
                    sync.dma_start(cand[t, :, :], cb[t][:, :]).then_inc(dma_c, 16)
                sync.wait_ge(dma_a, 16 * (CALLS // 2))
                sync.wait_ge(dma_b, 16 * (CALLS // 2))
                sync.wait_ge(dma_c, 16 * CALLS)

            @block.vector
            def _(vector):
                vector.memset(cinf[:, :], float("-inf")).then_inc(msem, 1)

            @block.scalar
            def _(scalar):
                scalar.wait_ge(msem, 1)
                for c in range(NFCH):
                    scalar.dma_start(
                        yf[:, c * FW : (c + 1) * FW], cinf[:, :]
                    ).then_inc(dma_y, 16)
                scalar.wait_ge(dma_y, 16 * NFCH)

            @block.gpsimd
            def _(g):
                for t in range(CALLS):
                    g.wait_ge(in_sems[t % 2], 16 * (t // 2 + 1))
                    g.topk(
                        cb[t][:, :],
                        xbufs[t % 2][:, :],
                        tokens=TOKS,
                        vocab_size=VH,
                        k=KTOP,
                    ).then_inc(comp, 1)

    nc.finalize()
    return nc


def _build_phase2():
    import concourse.mybir as mybir
    from concourse import bacc
    from concourse.tile import TileContext

    f32 = mybir.dt.float32
    alu = mybir.AluOpType

    nc = bacc.Bacc()
    x = nc.dram_tensor("x", [RPC, V], f32, kind="ExternalInput")
    vcut = nc.dram_tensor("vcut", [128, 1], f32, kind="ExternalInput")
    y = nc.dram_tensor("y", [RPC, V], f32, kind="ExternalOutput")

    xr = x[:, :].rearrange("r (p n) -> (r p) n", p=4)
    yr = y[:, :].rearrange("r (p n) -> (r p) n", p=4)

    with TileContext(nc) as tc:
        with (
            tc.tile_pool(name="xp", bufs=3) as xp,
            tc.tile_pool(name="mp", bufs=3) as mp,
            tc.tile_pool(name="vp", bufs=1) as vp,
        ):
            vt = vp.tile([128, 1], f32)
            nc.sync.dma_start(vt[:, :], vcut[:, :])
            for c in range(NCH):
                xt = xp.tile([128, F2], f32, tag="x")
                nc.sync.dma_start(xt[:, :], xr[:, c * F2 : (c + 1) * F2])
                mt = mp.tile([128, F2], f32, tag="m")
                # m = (x < vcut) * BIG   (2x DVE mode: single tensor source)
                nc.vector.tensor_scalar(
                    mt[:, :], xt[:, :], vt[:, :], BIG, alu.is_lt, alu.mult
                )
                # x = (m * 2) + x   -> x where kept, -inf where dropped
                nc.vector.scalar_tensor_tensor(
                    xt[:, :], mt[:, :], 2.0, xt[:, :], alu.mult, alu.add
                )
                nc.sync.dma_start(yr[:, c * F2 : (c + 1) * F2], xt[:, :])
    nc.finalize()
    return nc


def _get(name):
    if name not in _cache:
        _cache[name] = _build_phase1() if name == "p1" else _build_phase2()
    return _cache[name]


def _io_spec(nc):
    import concourse.mybir as mybir

    part = nc.partition_id_tensor.name if nc.partition_id_tensor else None
    ins, outs, avals = [], [], []
    for alloc in nc.m.functions[0].allocations:
        if not isinstance(alloc, mybir.MemoryLocationSet):
            continue
        name = alloc.memorylocations[0].name
        if alloc.kind == "ExternalInput":
            if name != part:
                ins.append(name)
        elif alloc.kind == "ExternalOutput":
            outs.append(name)
            avals.append(
                (tuple(alloc.tensor_shape), mybir.dt.np(alloc.dtype))
            )
    return part, ins, outs, avals


def _make_runner(nc):
    """Cached jitted 8-core shard_map runner for a finalized Bass module
    (mirrors bass2jax.run_bass_via_pjrt, but reusable across calls)."""
    import jax
    from jax.sharding import Mesh, PartitionSpec
    from jax.experimental.shard_map import shard_map
    from concourse import bass2jax

    bass2jax.install_neuronx_cc_hook()
    part, in_names, out_names, avals = _io_spec(nc)
    out_avals = [jax.core.ShapedArray(s, d) for s, d in avals]
    n_params = len(in_names)
    full_in_names = list(in_names) + list(out_names) + ([part] if part else [])
    donate = tuple(range(n_params, n_params + len(out_names)))

    def _body(*args):
        operands = list(args)
        if part is not None:
            operands.append(bass2jax.partition_id_tensor())
        outs = bass2jax._bass_exec_p.bind(
            *operands,
            out_avals=tuple(out_avals),
            in_names=tuple(full_in_names),
            out_names=tuple(out_names),
            lowering_input_output_aliases=(),
            sim_require_finite=True,
            sim_require_nnan=True,
            nc=nc,
        )
        return tuple(outs)

    devices = jax.devices()[:NCORES]
    mesh = Mesh(np.asarray(devices), ("core",))
    in_specs = (PartitionSpec("core"),) * (n_params + len(out_names))
    out_specs = (PartitionSpec("core"),) * len(out_names)
    sharded = jax.jit(
        shard_map(
            _body, mesh=mesh, in_specs=in_specs, out_specs=out_specs,
            check_rep=False,
        ),
        donate_argnums=donate,
        keep_unused=True,
    )

    def run(in_maps):
        concat_in = [
            np.concatenate([m[name] for m in in_maps], axis=0)
            for name in in_names
        ]
        concat_zeros = [
            np.zeros((NCORES * s[0], *s[1:]), d) for s, d in avals
        ]
        out_arrs = sharded(*concat_in, *concat_zeros)
        return [
            {
                name: np.asarray(out_arrs[i]).reshape(
                    NCORES, *avals[i][0]
                )[c]
                for i, name in enumerate(out_names)
            }
            for c in range(NCORES)
        ]

    return run


def _run_spmd(nc, in_maps, phase):
    """Run on the 8 NeuronCores; returns list of per-core output dicts."""
    if os.environ.get("BASS_KERNEL_SIM"):
        from concourse.bass_interp import CoreSim

        results = []
        for m in in_maps:
            sim = CoreSim(nc, require_finite=False)
            for k, v in m.items():
                sim.tensor(k)[:] = v
            sim.simulate()
            out = {}
            for alloc in nc.m.functions[0].allocations:
                try:
                    kind = alloc.kind
                except AttributeError:
                    continue
                if kind == "ExternalOutput":
                    name = alloc.memorylocations[0].name
                    out[name] = np.array(sim.tensor(name))
            last_exec_ns[phase] = int(sim.time)
            results.append(out)
        return results

    if os.environ.get("BASS_KERNEL_TRACE"):
        from concourse.bass_utils import run_bass_kernel_spmd

        res = run_bass_kernel_spmd(
            nc,
            in_maps,
            core_ids=list(range(NCORES)),
            trace=True,
            trace_cores=list(range(NCORES)),
            stitch_traces=False,
        )
        if res.exec_time_ns is not None:
            last_exec_ns[phase] = int(res.exec_time_ns)
        return res.results

    key = ("runner", phase)
    if key not in _cache:
        _cache[key] = _make_runner(nc)
    return _cache[key](in_maps)


def _decode_candidates(cands):
    """cands: per-core [CALLS,128,32] uint32 (one top-256 per half-row)
    -> per full row: vals [R, 2*KTOP] f32, idx [R, 2*KTOP] int64 (global)."""
    vals = np.empty((R, HALVES * KTOP), np.float32)
    idxs = np.empty((R, HALVES * KTOP), np.int64)
    half_off = np.tile(
        np.repeat(np.arange(HALVES) * VH, KTOP)[None, :], (RPC, 1)
    )
    for c, cd in enumerate(cands):
        blk = cd.reshape(CALLS, TOKS, 16, 32)
        v = np.ascontiguousarray(blk[..., :16]).reshape(VROWS, KTOP)
        i = np.ascontiguousarray(blk[..., 16:]).reshape(VROWS, KTOP)
        # virtual row v = 2*r + h  ->  row-major [RPC, HALVES*KTOP]
        vals[c * RPC : (c + 1) * RPC] = v.view(np.float32).reshape(
            RPC, HALVES * KTOP
        )
        idxs[c * RPC : (c + 1) * RPC] = (
            i.astype(np.int64).reshape(RPC, HALVES * KTOP) + half_off
        )
    return vals, idxs


def _jax_cpu():
    import jax

    return jax, jax.devices("cpu")[0]


def _window_decision(svals, kvec, top_ps):
    """Replicate the reference's f32 softmax/cumsum/top-p decision on the
    64-wide sorted window (bitwise-faithful to the full-width computation:
    the nonzero prefix occupies the same dyadic subtrees). Returns
    (n_keep [R], mask [R,TOPQ], boundary_margin [R])."""
    jax, cpu = _jax_cpu()
    import jax.numpy as jnp

    with jax.default_device(cpu):
        sv = jnp.asarray(svals)
        kk = jnp.asarray(kvec.astype(np.int32))
        tp = jnp.asarray(top_ps)
        ranks = jnp.arange(TOPQ, dtype=jnp.int32)
        mask_k = ranks[None, :] < kk[:, None]
        tl = jnp.where(mask_k, sv, -jnp.inf)
        probs = jax.nn.softmax(tl, axis=-1)
        cum = jnp.cumsum(probs, axis=-1)
        prev = cum - probs
        mask_p = prev <= tp[:, None]
        mask = mask_k & mask_p
        mask = mask.at[:, 0].set(True)
        n_keep = mask.sum(axis=-1).astype(jnp.int32)
        margin = jnp.min(
            jnp.where(mask_k, jnp.abs(prev - tp[:, None]), jnp.inf), axis=-1
        )
        return (
            np.asarray(n_keep),
            np.asarray(mask),
            np.asarray(margin),
        )


def _reference_rows(rows, logits, top_ks, top_ps, q):
    """Literal reference math (jnp f32, CPU) for a small set of rows.
    Returns (sel_idx [n] int32, sel_logits [n, V] f32)."""
    jax, cpu = _jax_cpu()
    import jax.numpy as jnp

    with jax.default_device(cpu):
        lg = jnp.asarray(logits[rows])
        k = jnp.asarray(top_ks[rows].astype(np.int32))
        tp = jnp.asarray(top_ps[rows])
        qq = jnp.asarray(q[rows])
        order = jnp.argsort(-lg, axis=-1)
        sorted_logits = jnp.take_along_axis(lg, order, axis=-1)
        k = jnp.where(k <= 0, V, k)
        ranks = jnp.arange(V, dtype=jnp.int32)
        mask_k = ranks[None, :] < k[:, None]
        topk_logits = jnp.where(mask_k, sorted_logits, -jnp.inf)
        probs = jax.nn.softmax(topk_logits, axis=-1)
        cum = jnp.cumsum(probs, axis=-1)
        mask_p = (cum - probs) <= tp[:, None]
        mask = mask_k & mask_p
        mask = mask.at[:, 0].set(True)
        masked_sorted = jnp.where(mask, sorted_logits, -jnp.inf)
        inv = jnp.argsort(order, axis=-1)
        selected_logits = jnp.take_along_axis(masked_sorted, inv, axis=-1)
        final_probs = jax.nn.softmax(selected_logits, axis=-1)
        sel = jnp.argmax(final_probs / (qq + EPS), axis=-1).astype(jnp.int32)
        return np.asarray(sel), np.asarray(selected_logits)


def _reference_sample_rows(rows, sel_logits, q):
    """Exact reference sampling (argmax of softmax(sel_logits)/(q+eps))
    for specific rows, given the final selected_logits."""
    jax, cpu = _jax_cpu()
    import jax.numpy as jnp

    with jax.default_device(cpu):
        sl = jnp.asarray(sel_logits[rows])
        qq = jnp.asarray(q[rows])
        fp = jax.nn.softmax(sl, axis=-1)
        return np.asarray(jnp.argmax(fp / (qq + EPS), axis=-1).astype(jnp.int32))


def kernel(logits, top_ks, top_ps, q):
    logits = np.ascontiguousarray(np.asarray(logits, dtype=np.float32))
    kvec = np.asarray(top_ks).astype(np.int64).reshape(R)
    top_ps = np.asarray(top_ps, dtype=np.float32).reshape(R)
    q = np.asarray(q, dtype=np.float32)

    # ---- phase 1: exact top-256 per row on device ----
    nc1 = _get("p1")
    in1 = [{"x": logits[c * RPC : (c + 1) * RPC]} for c in range(NCORES)]
    r1 = _run_spmd(nc1, in1, "p1")
    vals_asc, idxs_asc = _decode_candidates([r["cand"] for r in r1])

    # top-64 in reference order: value desc, index asc (stable ties)
    ordw = np.lexsort((idxs_asc, -vals_asc), axis=-1)[:, :TOPQ]
    rows_i = np.arange(R)[:, None]
    svals = np.take_along_axis(vals_asc, ordw, 1)
    sidx = np.take_along_axis(idxs_asc, ordw, 1)

    bad = np.zeros(R, dtype=bool)
    # device-value sanity: values must equal logits at the reported indices
    bad |= np.any(logits[rows_i, sidx] != svals, axis=1)
    # duplicate indices within a row's top-64 (ucode tie pathology)
    ss = np.sort(sidx, axis=1)
    bad |= np.any(ss[:, 1:] == ss[:, :-1], axis=1)
    # k outside the top-64 window
    bad |= (kvec <= 0) | (kvec >= TOPQ)

    # ---- host decision: n_keep / v_cut per row (f32, reference-faithful) ----
    kk = np.where(bad, 1, kvec).astype(np.int64)
    n_keep, mask, margin = _window_decision(svals, kk, top_ps)
    # non-prefix mask would break the threshold construction
    prefix = np.arange(TOPQ)[None, :] < n_keep[:, None]
    bad |= np.any(mask != prefix, axis=1)
    bad |= margin < 1e-5
    n_keep = np.clip(n_keep, 1, TOPQ)
    v_cut = np.take_along_axis(svals, n_keep[:, None] - 1, 1)[:, 0]

    # ---- phase 2: threshold-materialize the output on device ----
    nc2 = _get("p2")
    in2 = []
    for c in range(NCORES):
        vc = np.repeat(v_cut[c * RPC : (c + 1) * RPC], 4).reshape(128, 1)
        vc = np.ascontiguousarray(vc.astype(np.float32))
        in2.append({"x": logits[c * RPC : (c + 1) * RPC], "vcut": vc})
    r2 = _run_spmd(nc2, in2, "p2")
    out = np.concatenate([r["y"] for r in r2], axis=0)

    # tie fix: elements equal to v_cut beyond the kept prefix -> -inf
    beyond = np.arange(TOPQ)[None, :] >= n_keep[:, None]
    tiefix = beyond & (svals == v_cut[:, None])
    for r_ in np.nonzero(np.any(tiefix, axis=1))[0]:
        if not bad[r_]:
            out[r_, sidx[r_, tiefix[r_]]] = NEG_INF

    # ---- sampling: argmax p/(q+eps) over kept positions (f64 + guard) ----
    kept = ~beyond
    sv64 = svals.astype(np.float64)
    e = np.exp(sv64 - sv64[:, :1]) * kept
    p = e / e.sum(axis=1, keepdims=True)
    qg = q[rows_i, sidx]
    den = (qg + np.float32(EPS)).astype(np.float64)
    ratio = np.where(kept, p / den, -1.0)
    best = ratio.max(axis=1)
    # winner = lowest vocab index among exact-max ties
    is_best = ratio == best[:, None]
    sel_idx = np.where(is_best, sidx, np.int64(V + 1)).min(axis=1).astype(np.int32)
    # near-tie guard: second-distinct ratio too close -> exact fallback
    second = np.where(is_best, -np.inf, ratio).max(axis=1)
    with np.errstate(invalid="ignore", divide="ignore"):
        close = (best - second) <= 1e-5 * np.abs(best)
    n_best = is_best.sum(axis=1)
    sample_rows = np.nonzero((close | (n_best > 1)) & ~bad)[0]
    if sample_rows.size:
        sel_idx[sample_rows] = _reference_sample_rows(sample_rows, out, q)

    # ---- full fallback for anomalous rows (expected: none) ----
    bad_rows = np.nonzero(bad)[0]
    if bad_rows.size:
        fb_idx, fb_logits = _reference_rows(bad_rows, logits, kvec, top_ps, q)
        sel_idx[bad_rows] = fb_idx
        out[bad_rows] = fb_logits

    return sel_idx.astype(np.int32), out.astype(np.float32, copy=False)


# revision 28
# speedup vs baseline: 9.7782x; 9.7782x over previous
"""Fused top-k/top-p/sampling kernel for Trainium2 (8 NeuronCores).

Contract: kernel(**inputs) takes FULL inputs (logits [256,128000] f32,
top_ks [256] int, top_ps [256] f32, q [256,128000] f32) and returns the
FULL output tuple (selected_idx [256] int32, selected_logits [256,128000]
f32), matching reference semantics.

Strategy (rows sharded 32/core across 8 cores, pure data parallel):
  Phase 1 (device): per row, exact top-256 (values + indices) via the
      gpsimd `topk` ucode instruction. Only ~16KB/core comes back.
  Host glue (O(rows*64)): stable top-64 order, replicate the reference's
      f32 top-k/top-p prefix decision on the 64-wide window, derive the
      per-row value cutoff v_cut and the sampled index (argmax of
      p/(q+eps) over the <=63 kept positions, gathering q on host).
  Phase 2 (device): stream logits again and write
      out = x + 2*((x < v_cut) * -1.8e38)
      which is exactly x where x >= v_cut and exactly -inf elsewhere
      (the *2 overflows -3.6e38 to -inf in f32; 0*2+x == x is exact).
      Tie rows (elements == v_cut beyond the kept prefix) are patched on
      host using the exact candidate info from phase 1.

The decision math only needs the top-63 elements per row (top_ks < 64),
so q is never streamed on device and the device does two 16MB/core reads
plus one 16MB/core write: ~393MB of HBM traffic total across 8 cores.
"""

import os
import sys

if "/opt/trn_rl_repo" not in sys.path:
    sys.path.insert(0, "/opt/trn_rl_repo")

import numpy as np

R, V = 256, 128000
NCORES = 8
RPC = R // NCORES          # 32 rows per core
HALVES = 2                 # topk ISA encodes n as u16 -> split rows in half
VH = V // HALVES           # 64000 per half-row "token"
VROWS = RPC * HALVES       # 64 virtual tokens per core
TOKS = 8                   # tokens per topk instruction
CALLS = VROWS // TOKS      # 8 topk calls per core
KTOP = 256                 # candidates per half-row from device
TOPQ = 64                  # candidates actually used per row (top_ks < 64)
F2 = 2000                  # phase-2 free-dim tile size (per partition)
NCH = V // 4 // F2         # 16 chunks (4 partitions per row)
BIG = -1.8e38              # (x<t)*BIG*2 + x  ->  -inf exactly when dropped
EPS = 1e-08
NEG_INF = np.float32(-np.inf)

_cache = {}
last_exec_ns = {}          # phase -> exec_time_ns (filled when tracing)


def _build_phase1():
    from contextlib import ExitStack

    import concourse.mybir as mybir
    from concourse import bacc

    nc = bacc.Bacc()
    x = nc.dram_tensor("x", [RPC, V], mybir.dt.float32, kind="ExternalInput")
    cand = nc.dram_tensor(
        "cand", [CALLS, 128, 32], mybir.dt.uint32, kind="ExternalOutput"
    )
    with (
        nc.semaphore("dma_a") as dma_a,
        nc.semaphore("dma_b") as dma_b,
        nc.semaphore("dma_c") as dma_c,
        nc.semaphore("comp") as comp,
        nc.sbuf_tensor("xb0", [128, VH // 16], mybir.dt.float32) as xb0,
        nc.sbuf_tensor("xb1", [128, VH // 16], mybir.dt.float32) as xb1,
        ExitStack() as stack,
    ):
        cb = [
            stack.enter_context(
                nc.sbuf_tensor(f"cb{t}", [128, 32], mybir.dt.uint32)
            )
            for t in range(CALLS)
        ]
        xbufs = [xb0, xb1]
        in_sems = [dma_a, dma_b]
        # rows x halves as 64 virtual tokens of width VH (contiguous)
        xv = x[:, :].rearrange("r (h n) -> (r h) n", h=HALVES)

        with nc.Block() as block:

            @block.sync
            def _(sync):
                for t in range(CALLS):
                    if t >= 2:
                        # buffer t%2 is free once topk t-2 completed
                        sync.wait_ge(comp, t - 1)
                    sync.dma_start(
                        xbufs[t % 2][:, :],
                        xv[t * TOKS : (t + 1) * TOKS, :].rearrange(
                            "v (p c) -> (v p) c", p=16
                        ),
                    ).then_inc(in_sems[t % 2], 16)
                for t in range(CALLS):
                    sync.wait_ge(comp, t + 1)
                    sync.dma_start(cand[t, :, :], cb[t][:, :]).then_inc(dma_c, 16)
                sync.wait_ge(dma_a, 16 * (CALLS // 2))
                sync.wait_ge(dma_b, 16 * (CALLS // 2))
                sync.wait_ge(dma_c, 16 * CALLS)

            @block.gpsimd
            def _(g):
                for t in range(CALLS):
                    g.wait_ge(in_sems[t % 2], 16 * (t // 2 + 1))
                    g.topk(
                        cb[t][:, :],
                        xbufs[t % 2][:, :],
                        tokens=TOKS,
                        vocab_size=VH,
                        k=KTOP,
                    ).then_inc(comp, 1)

    nc.finalize()
    return nc


RT = 4                      # rows per input tile in the fused kernel
NT = RPC // RT              # 8 input tiles per core
CH = V // 128               # 1000: per-partition chunk of one row
NC8 = 8                     # DVE max8 width


def _build_fused():
    """Single launch per core:
      - sync: stream logits in (8 x 2MB tiles, 4 rows each, double-buffered)
      - vector (DVE): per row, `max` (top-8 values per partition) +
        `max_index` (their indices) over the [128, 1000] row view
      - gpsimd: memset a [128, 4000] -inf tile
      - scalar: DMA the -inf tile over the whole output (write-only fill,
        overlaps the input reads)
    Host later scatters the <=63 kept logits per row into the output and
    derives everything else from the 1024 (value, index) candidates.
    """
    import concourse.mybir as mybir
    from concourse import bacc

    FW = 4000
    NFCH = V * RPC // (128 * FW)  # 8 fill chunks

    nc = bacc.Bacc()
    x = nc.dram_tensor("x", [RPC, V], mybir.dt.float32, kind="ExternalInput")
    cv = nc.dram_tensor(
        "cv", [128, RPC * NC8], mybir.dt.float32, kind="ExternalOutput"
    )
    ci = nc.dram_tensor(
        "ci", [128, RPC * NC8], mybir.dt.uint32, kind="ExternalOutput"
    )
    y = nc.dram_tensor("y", [RPC, V], mybir.dt.float32, kind="ExternalOutput")

    with (
        nc.semaphore("dma_a") as dma_a,
        nc.semaphore("dma_b") as dma_b,
        nc.semaphore("dma_c") as dma_c,
        nc.semaphore("dma_y") as dma_y,
        nc.semaphore("comp") as comp,
        nc.semaphore("msem") as msem,
        nc.sbuf_tensor("xb0", [128, RT * CH], mybir.dt.float32) as xb0,
        nc.sbuf_tensor("xb1", [128, RT * CH], mybir.dt.float32) as xb1,
        nc.sbuf_tensor("cvb", [128, RPC * NC8], mybir.dt.float32) as cvb,
        nc.sbuf_tensor("cib", [128, RPC * NC8], mybir.dt.uint32) as cib,
        nc.sbuf_tensor("cinf", [128, FW], mybir.dt.float32) as cinf,
    ):
        xbufs = [xb0, xb1]
        in_sems = [dma_a, dma_b]
        # input tile t: rows 4t..4t+4; partition p gets, for each row,
        # that row's elements [p*CH, (p+1)*CH)
        xt = x[:, :].rearrange("(t r) (p c) -> t p r c", t=NT, p=128)
        # output fill view: partition = 4*row + quarter, 32000 contiguous
        yf = y[:, :].rearrange("r (a n) -> (r a) n", a=4)

        with nc.Block() as block:

            @block.sync
            def _(sync):
                for t in range(NT):
                    if t >= 2:
                        # xbuf t%2 free once DVE finished tile t-2
                        sync.wait_ge(comp, 2 * RT * (t - 1))
                    sync.dma_start(xbufs[t % 2][:, :], xt[t, :, :, :]).then_inc(
                        in_sems[t % 2], 16
                    )
                sync.wait_ge(comp, 2 * RT * NT)
                sync.dma_start(cv[:, :], cvb[:, :]).then_inc(dma_c, 16)
                sync.dma_start(ci[:, :], cib[:, :]).then_inc(dma_c, 16)
                sync.wait_ge(dma_a, 16 * (NT // 2))
                sync.wait_ge(dma_b, 16 * (NT // 2))
                sync.wait_ge(dma_c, 32)

            @block.vector
            def _(vector):
                for t in range(NT):
                    vector.wait_ge(in_sems[t % 2], 16 * (t // 2 + 1))
                    xb = xbufs[t % 2]
                    for rr in range(RT):
                        r = t * RT + rr
                        vector.max(
                            cvb[:, r * NC8 : (r + 1) * NC8],
                            xb[:, rr * CH : (rr + 1) * CH],
                        ).then_inc(comp, 1)
                        vector.wait_ge(comp, 2 * r + 1)
                        vector.max_index(
                            cib[:, r * NC8 : (r + 1) * NC8],
                            cvb[:, r * NC8 : (r + 1) * NC8],
                            xb[:, rr * CH : (rr + 1) * CH],
                        ).then_inc(comp, 1)

            @block.gpsimd
            def _(g):
                g.memset(cinf[:, :], float("-inf")).then_inc(msem, 1)

            @block.scalar
            def _(scalar):
                scalar.wait_ge(msem, 1)
                for c in range(NFCH):
                    scalar.dma_start(
                        yf[:, c * FW : (c + 1) * FW], cinf[:, :]
                    ).then_inc(dma_y, 16)
                scalar.wait_ge(dma_y, 16 * NFCH)

    nc.finalize()
    return nc


def _build_phase2():
    import concourse.mybir as mybir
    from concourse import bacc
    from concourse.tile import TileContext

    f32 = mybir.dt.float32
    alu = mybir.AluOpType

    nc = bacc.Bacc()
    x = nc.dram_tensor("x", [RPC, V], f32, kind="ExternalInput")
    vcut = nc.dram_tensor("vcut", [128, 1], f32, kind="ExternalInput")
    y = nc.dram_tensor("y", [RPC, V], f32, kind="ExternalOutput")

    xr = x[:, :].rearrange("r (p n) -> (r p) n", p=4)
    yr = y[:, :].rearrange("r (p n) -> (r p) n", p=4)

    with TileContext(nc) as tc:
        with (
            tc.tile_pool(name="xp", bufs=3) as xp,
            tc.tile_pool(name="mp", bufs=3) as mp,
            tc.tile_pool(name="vp", bufs=1) as vp,
        ):
            vt = vp.tile([128, 1], f32)
            nc.sync.dma_start(vt[:, :], vcut[:, :])
            for c in range(NCH):
                xt = xp.tile([128, F2], f32, tag="x")
                nc.sync.dma_start(xt[:, :], xr[:, c * F2 : (c + 1) * F2])
                mt = mp.tile([128, F2], f32, tag="m")
                # m = (x < vcut) * BIG   (2x DVE mode: single tensor source)
                nc.vector.tensor_scalar(
                    mt[:, :], xt[:, :], vt[:, :], BIG, alu.is_lt, alu.mult
                )
                # x = (m * 2) + x   -> x where kept, -inf where dropped
                nc.vector.scalar_tensor_tensor(
                    xt[:, :], mt[:, :], 2.0, xt[:, :], alu.mult, alu.add
                )
                nc.sync.dma_start(yr[:, c * F2 : (c + 1) * F2], xt[:, :])
    nc.finalize()
    return nc


_builders = {
    "p1": _build_phase1,
    "p2": _build_phase2,
    "fused": _build_fused,
}


def _get(name):
    if name not in _cache:
        _cache[name] = _builders[name]()
    return _cache[name]


def _io_spec(nc):
    import concourse.mybir as mybir

    part = nc.partition_id_tensor.name if nc.partition_id_tensor else None
    ins, outs, avals = [], [], []
    for alloc in nc.m.functions[0].allocations:
        if not isinstance(alloc, mybir.MemoryLocationSet):
            continue
        name = alloc.memorylocations[0].name
        if alloc.kind == "ExternalInput":
            if name != part:
                ins.append(name)
        elif alloc.kind == "ExternalOutput":
            outs.append(name)
            avals.append(
                (tuple(alloc.tensor_shape), mybir.dt.np(alloc.dtype))
            )
    return part, ins, outs, avals


def _make_runner(nc):
    """Cached jitted 8-core shard_map runner for a finalized Bass module
    (mirrors bass2jax.run_bass_via_pjrt, but reusable across calls)."""
    import jax
    from jax.sharding import Mesh, PartitionSpec
    from jax.experimental.shard_map import shard_map
    from concourse import bass2jax

    bass2jax.install_neuronx_cc_hook()
    part, in_names, out_names, avals = _io_spec(nc)
    out_avals = [jax.core.ShapedArray(s, d) for s, d in avals]
    n_params = len(in_names)
    full_in_names = list(in_names) + list(out_names) + ([part] if part else [])
    donate = tuple(range(n_params, n_params + len(out_names)))

    def _body(*args):
        operands = list(args)
        if part is not None:
            operands.append(bass2jax.partition_id_tensor())
        outs = bass2jax._bass_exec_p.bind(
            *operands,
            out_avals=tuple(out_avals),
            in_names=tuple(full_in_names),
            out_names=tuple(out_names),
            lowering_input_output_aliases=(),
            sim_require_finite=True,
            sim_require_nnan=True,
            nc=nc,
        )
        return tuple(outs)

    devices = jax.devices()[:NCORES]
    mesh = Mesh(np.asarray(devices), ("core",))
    in_specs = (PartitionSpec("core"),) * (n_params + len(out_names))
    out_specs = (PartitionSpec("core"),) * len(out_names)
    sharded = jax.jit(
        shard_map(
            _body, mesh=mesh, in_specs=in_specs, out_specs=out_specs,
            check_rep=False,
        ),
        donate_argnums=donate,
        keep_unused=True,
    )

    def run(in_maps):
        concat_in = [
            np.concatenate([m[name] for m in in_maps], axis=0)
            for name in in_names
        ]
        concat_zeros = [
            np.zeros((NCORES * s[0], *s[1:]), d) for s, d in avals
        ]
        out_arrs = sharded(*concat_in, *concat_zeros)
        return [
            {
                name: np.asarray(out_arrs[i]).reshape(
                    NCORES, *avals[i][0]
                )[c]
                for i, name in enumerate(out_names)
            }
            for c in range(NCORES)
        ]

    return run


def _run_spmd(nc, in_maps, phase):
    """Run on the 8 NeuronCores; returns list of per-core output dicts."""
    if os.environ.get("BASS_KERNEL_SIM"):
        from concourse.bass_interp import CoreSim

        results = []
        for m in in_maps:
            sim = CoreSim(nc, require_finite=False)
            for k, v in m.items():
                sim.tensor(k)[:] = v
            sim.simulate()
            out = {}
            for alloc in nc.m.functions[0].allocations:
                try:
                    kind = alloc.kind
                except AttributeError:
                    continue
                if kind == "ExternalOutput":
                    name = alloc.memorylocations[0].name
                    out[name] = np.array(sim.tensor(name))
            last_exec_ns[phase] = int(sim.time)
            results.append(out)
        return results

    if os.environ.get("BASS_KERNEL_TRACE"):
        from concourse.bass_utils import run_bass_kernel_spmd

        res = run_bass_kernel_spmd(
            nc,
            in_maps,
            core_ids=list(range(NCORES)),
            trace=True,
            trace_cores=list(range(NCORES)),
            stitch_traces=False,
        )
        if res.exec_time_ns is not None:
            last_exec_ns[phase] = int(res.exec_time_ns)
        return res.results

    key = ("runner", phase)
    if key not in _cache:
        _cache[key] = _make_runner(nc)
    return _cache[key](in_maps)


def _decode_candidates(cands):
    """cands: per-core [CALLS,128,32] uint32 (one top-256 per half-row)
    -> per full row: vals [R, 2*KTOP] f32, idx [R, 2*KTOP] int64 (global)."""
    vals = np.empty((R, HALVES * KTOP), np.float32)
    idxs = np.empty((R, HALVES * KTOP), np.int64)
    half_off = np.tile(
        np.repeat(np.arange(HALVES) * VH, KTOP)[None, :], (RPC, 1)
    )
    for c, cd in enumerate(cands):
        blk = cd.reshape(CALLS, TOKS, 16, 32)
        v = np.ascontiguousarray(blk[..., :16]).reshape(VROWS, KTOP)
        i = np.ascontiguousarray(blk[..., 16:]).reshape(VROWS, KTOP)
        # virtual row v = 2*r + h  ->  row-major [RPC, HALVES*KTOP]
        vals[c * RPC : (c + 1) * RPC] = v.view(np.float32).reshape(
            RPC, HALVES * KTOP
        )
        idxs[c * RPC : (c + 1) * RPC] = (
            i.astype(np.int64).reshape(RPC, HALVES * KTOP) + half_off
        )
    return vals, idxs


def _decode_candidates_fused(results):
    """Per-core {'cv': [128, RPC*8] f32, 'ci': [128, RPC*8] u32} ->
    vals [R, 1024] f32, idx [R, 1024] int64 (global vocab indices)."""
    vals = np.empty((R, 128 * NC8), np.float32)
    idxs = np.empty((R, 128 * NC8), np.int64)
    poff = (np.arange(128, dtype=np.int64) * CH)[:, None, None]
    for c, res in enumerate(results):
        cvc = res["cv"].reshape(128, RPC, NC8)
        cic = res["ci"].astype(np.int64).reshape(128, RPC, NC8) + poff
        vals[c * RPC : (c + 1) * RPC] = (
            cvc.transpose(1, 0, 2).reshape(RPC, 128 * NC8)
        )
        idxs[c * RPC : (c + 1) * RPC] = (
            cic.transpose(1, 0, 2).reshape(RPC, 128 * NC8)
        )
    return vals, idxs


def _jax_cpu():
    import jax

    return jax, jax.devices("cpu")[0]


def _window_decision(svals, kvec, top_ps):
    """Replicate the reference's f32 softmax/cumsum/top-p decision on the
    64-wide sorted window (bitwise-faithful to the full-width computation:
    the nonzero prefix occupies the same dyadic subtrees). Returns
    (n_keep [R], mask [R,TOPQ], boundary_margin [R])."""
    jax, cpu = _jax_cpu()
    import jax.numpy as jnp

    with jax.default_device(cpu):
        sv = jnp.asarray(svals)
        kk = jnp.asarray(kvec.astype(np.int32))
        tp = jnp.asarray(top_ps)
        ranks = jnp.arange(TOPQ, dtype=jnp.int32)
        mask_k = ranks[None, :] < kk[:, None]
        tl = jnp.where(mask_k, sv, -jnp.inf)
        probs = jax.nn.softmax(tl, axis=-1)
        cum = jnp.cumsum(probs, axis=-1)
        prev = cum - probs
        mask_p = prev <= tp[:, None]
        mask = mask_k & mask_p
        mask = mask.at[:, 0].set(True)
        n_keep = mask.sum(axis=-1).astype(jnp.int32)
        margin = jnp.min(
            jnp.where(mask_k, jnp.abs(prev - tp[:, None]), jnp.inf), axis=-1
        )
        return (
            np.asarray(n_keep),
            np.asarray(mask),
            np.asarray(margin),
        )


def _reference_rows(rows, logits, top_ks, top_ps, q):
    """Literal reference math (jnp f32, CPU) for a small set of rows.
    Returns (sel_idx [n] int32, sel_logits [n, V] f32)."""
    jax, cpu = _jax_cpu()
    import jax.numpy as jnp

    with jax.default_device(cpu):
        lg = jnp.asarray(logits[rows])
        k = jnp.asarray(top_ks[rows].astype(np.int32))
        tp = jnp.asarray(top_ps[rows])
        qq = jnp.asarray(q[rows])
        order = jnp.argsort(-lg, axis=-1)
        sorted_logits = jnp.take_along_axis(lg, order, axis=-1)
        k = jnp.where(k <= 0, V, k)
        ranks = jnp.arange(V, dtype=jnp.int32)
        mask_k = ranks[None, :] < k[:, None]
        topk_logits = jnp.where(mask_k, sorted_logits, -jnp.inf)
        probs = jax.nn.softmax(topk_logits, axis=-1)
        cum = jnp.cumsum(probs, axis=-1)
        mask_p = (cum - probs) <= tp[:, None]
        mask = mask_k & mask_p
        mask = mask.at[:, 0].set(True)
        masked_sorted = jnp.where(mask, sorted_logits, -jnp.inf)
        inv = jnp.argsort(order, axis=-1)
        selected_logits = jnp.take_along_axis(masked_sorted, inv, axis=-1)
        final_probs = jax.nn.softmax(selected_logits, axis=-1)
        sel = jnp.argmax(final_probs / (qq + EPS), axis=-1).astype(jnp.int32)
        return np.asarray(sel), np.asarray(selected_logits)


def _reference_sample_rows(rows, sel_logits, q):
    """Exact reference sampling (argmax of softmax(sel_logits)/(q+eps))
    for specific rows, given the final selected_logits."""
    jax, cpu = _jax_cpu()
    import jax.numpy as jnp

    with jax.default_device(cpu):
        sl = jnp.asarray(sel_logits[rows])
        qq = jnp.asarray(q[rows])
        fp = jax.nn.softmax(sl, axis=-1)
        return np.asarray(jnp.argmax(fp / (qq + EPS), axis=-1).astype(jnp.int32))


def kernel(logits, top_ks, top_ps, q):
    logits = np.ascontiguousarray(np.asarray(logits, dtype=np.float32))
    kvec = np.asarray(top_ks).astype(np.int64).reshape(R)
    top_ps = np.asarray(top_ps, dtype=np.float32).reshape(R)
    q = np.asarray(q, dtype=np.float32)

    # ---- single fused launch: candidates + -inf-filled output ----
    ncf = _get("fused")
    in1 = [{"x": logits[c * RPC : (c + 1) * RPC]} for c in range(NCORES)]
    r1 = _run_spmd(ncf, in1, "fused")
    cvals, cidxs = _decode_candidates_fused(r1)
    out = np.concatenate([r["y"] for r in r1], axis=0)

    # top-64 in reference order: value desc, index asc (stable ties)
    ordw = np.lexsort((cidxs, -cvals), axis=-1)[:, :TOPQ]
    rows_i = np.arange(R)[:, None]
    svals = np.take_along_axis(cvals, ordw, 1)
    sidx = np.take_along_axis(cidxs, ordw, 1)

    bad = np.zeros(R, dtype=bool)
    # device-value sanity: values must equal logits at the reported indices
    bad |= np.any(logits[rows_i, sidx] != svals, axis=1)
    # duplicate candidate index whose value reaches the top-64 window
    # (max_index duplicate-needle pathology within one partition)
    iord = np.argsort(cidxs, axis=1)
    ci_s = np.take_along_axis(cidxs, iord, 1)
    cv_s = np.take_along_axis(cvals, iord, 1)
    v64 = svals[:, TOPQ - 1]
    dup = (ci_s[:, 1:] == ci_s[:, :-1]) & (cv_s[:, 1:] >= v64[:, None])
    bad |= np.any(dup, axis=1)
    # a partition that filled all 8 max8 slots inside the top-64 may be
    # hiding a 9th element that also belongs there
    p64 = (sidx // CH).astype(np.int64)
    pcnt = np.zeros((R, 128), np.int64)
    np.add.at(pcnt, (np.broadcast_to(rows_i, p64.shape), p64), 1)
    bad |= np.any(pcnt >= NC8, axis=1)
    # k outside the top-64 window
    bad |= (kvec <= 0) | (kvec >= TOPQ)

    # ---- host decision: n_keep per row (f32, reference-faithful) ----
    kk = np.where(bad, 1, kvec).astype(np.int64)
    n_keep, mask, margin = _window_decision(svals, kk, top_ps)
    # non-prefix mask would break the prefix-scatter construction
    prefix = np.arange(TOPQ)[None, :] < n_keep[:, None]
    bad |= np.any(mask != prefix, axis=1)
    bad |= margin < 1e-5
    n_keep = np.clip(n_keep, 1, TOPQ)

    # ---- scatter the kept logits into the -inf canvas ----
    keep = prefix & ~bad[:, None]
    out[np.broadcast_to(rows_i, keep.shape)[keep], sidx[keep]] = svals[keep]

    # ---- sampling: argmax p/(q+eps) over kept positions (f64 + guard) ----
    kept = prefix
    sv64 = svals.astype(np.float64)
    e = np.exp(sv64 - sv64[:, :1]) * kept
    p = e / e.sum(axis=1, keepdims=True)
    qg = q[rows_i, sidx]
    den = (qg + np.float32(EPS)).astype(np.float64)
    ratio = np.where(kept, p / den, -1.0)
    best = ratio.max(axis=1)
    # winner = lowest vocab index among exact-max ties
    is_best = ratio == best[:, None]
    sel_idx = np.where(is_best, sidx, np.int64(V + 1)).min(axis=1).astype(np.int32)
    # near-tie guard: second-distinct ratio too close -> exact fallback
    second = np.where(is_best, -np.inf, ratio).max(axis=1)
    with np.errstate(invalid="ignore", divide="ignore"):
        close = (best - second) <= 1e-5 * np.abs(best)
    n_best = is_best.sum(axis=1)
    sample_rows = np.nonzero((close | (n_best > 1)) & ~bad)[0]
    if sample_rows.size:
        sel_idx[sample_rows] = _reference_sample_rows(sample_rows, out, q)

    # ---- full fallback for anomalous rows (expected: none) ----
    bad_rows = np.nonzero(bad)[0]
    if bad_rows.size:
        fb_idx, fb_logits = _reference_rows(bad_rows, logits, kvec, top_ps, q)
        sel_idx[bad_rows] = fb_idx
        out[bad_rows] = fb_logits

    return sel_idx.astype(np.int32), out.astype(np.float32, copy=False)
